# revision 14
# baseline (speedup 1.0000x reference)
"""Trainium2 Bass kernel for a dense transformer block (B=4, S=2048, E=1024,
H=16 heads, DK=64, FFN 4x) distributed over 8 NeuronCores.

Sharding (fully uniform SPMD, no collectives):
  core c -> batch b = c//2, parity j = c%2.
  The core owns query/FFN tokens at positions j::2 of sequence b (1024
  tokens) and computes K/V over all 2048 tokens of b (duplicated across the
  two cores of a batch; cheaper than a cross-core collective).

Layout: activations are feature-major ([feature, token]) so every matmul
contracts over the partition dim with weights in natural layout.  The host
passes x[b].T and the strided query slice x[b][j::2].T, and transposes the
kernel output back.

Attention: scores are computed transposed ([keys, queries]) per head.
Softmax skips the max-subtraction (scores here have std ~0.1; exp is safe).
Row sums come from a ones-column inside the AV matmul (appended for even
heads, prepended for odd heads so each head's output lands on its own
partition range -- engines cannot move data across partitions).
Causal masking multiplies the exp'd scores with a per-core 0/1 mask tile.
"""

import sys

for _p in ("/opt/trn_rl_repo", "/opt/pypackages"):
    if _p not in sys.path:
        sys.path.append(_p)

import numpy as np
import ml_dtypes

import concourse.bass as bass
import concourse.mybir as mybir
import concourse.tile as tile
from concourse import bacc, bass_utils

F32 = mybir.dt.float32
BF16 = mybir.dt.bfloat16
BF = ml_dtypes.bfloat16
MUL = mybir.AluOpType.mult
ADD = mybir.AluOpType.add
SUB = mybir.AluOpType.subtract
AF = mybir.ActivationFunctionType

P = 128
S = 2048          # full sequence
TOK = 1024        # own tokens per core
E = 1024          # model dim (= D)
EO = E // P       # 8 feature subtiles
H = 16            # heads
DK = 64
FF = 4096
FO = FF // P      # 32
KT = S // P       # 16 key tiles
NG = TOK // 256   # 4 query groups
GQ = 256
CS = S // 512     # 4 chunks of 512 over full seq
CT = TOK // 512   # 2 chunks over own tokens
EPS = 1e-5

_PROG = None


def _emit_ln(nc, tc, pools, x_f32, gb_row, ncols, h_out, tag):
    """LayerNorm in feature-major layout.

    x_f32: SBUF [128, EO, ncols] fp32; gb_row: SBUF [2, E] bf16 (g, b);
    h_out: SBUF [128, EO, ncols] bf16.
    Per-token stats via ones-matmuls over the partition dim; per-token
    broadcast factors materialized by rank-1/rank-2 PE matmuls.
    """
    tmp = pools["tmp"]
    onesf, onesb = pools["ones_f"], pools["ones_b"]
    nch = ncols // 512

    with tc.tile_pool(name=f"lnr_{tag}", bufs=1, side="right") as rows:
        r1 = rows.tile([1, ncols], F32, tag="r1")
        r2 = rows.tile([1, ncols], F32, tag="r2")
        r3 = rows.tile([1, ncols], F32, tag="r3")
        rstd_bf = rows.tile([1, ncols], BF16, tag="rstdbf")
        mr = rows.tile([2, ncols], BF16, tag="mr")
        nc.vector.memset(mr[:, :], 1.0)  # row1 stays ones; row0 overwritten

        with tc.tile_pool(name=f"lnsq_{tag}", bufs=3, side="right") as sqp, \
             tc.tile_pool(name=f"lnst_{tag}", bufs=4, space="PSUM") as pst:
            for c in range(nch):
                sl = slice(c * 512, (c + 1) * 512)
                ps_sum = pst.tile([1, 512], F32, tag="st")
                for o in range(EO):
                    nc.tensor.matmul(ps_sum[:], onesf[:], x_f32[:, o, sl],
                                     start=(o == 0), stop=(o == EO - 1))
                nc.vector.tensor_copy(r1[:, sl], ps_sum[:])
                ps_sq = pst.tile([1, 512], F32, tag="st")
                for o in range(EO):
                    sq = sqp.tile([P, 512], BF16, tag="sq")
                    nc.vector.tensor_tensor(sq[:], x_f32[:, o, sl],
                                            x_f32[:, o, sl], MUL)
                    nc.tensor.matmul(ps_sq[:], onesb[:, 0:1], sq[:],
                                     start=(o == 0), stop=(o == EO - 1))
                nc.vector.tensor_copy(r2[:, sl], ps_sq[:])

        # r1=sum, r2=sumsq -> r3=mu, r2=rstd, mr[0]=-mu*rstd
        nc.vector.tensor_scalar_mul(r3[:], r1[:], 1.0 / E)
        nc.vector.tensor_tensor(r1[:], r3[:], r1[:], MUL)
        nc.vector.tensor_tensor(r1[:], r2[:], r1[:], SUB)
        nc.vector.tensor_scalar(r1[:], r1[:], 1.0 / E, EPS, MUL, ADD)
        nc.vector.reciprocal(r1[:], r1[:])
        nc.scalar.activation(r2[:], r1[:], AF.Sqrt)
        nc.vector.tensor_copy(rstd_bf[:], r2[:])
        nc.vector.tensor_tensor(r1[:], r3[:], r2[:], MUL)
        nc.vector.tensor_scalar_mul(mr[0:1, :], r1[:], -1.0)

        # h = x * (g[e]*rstd[t]) + (-mu[t]*rstd[t]*g[e] + b[e])
        with tc.tile_pool(name=f"lnbc_{tag}", bufs=4, space="PSUM") as pbc:
            for o in range(EO):
                osl = slice(o * P, (o + 1) * P)
                for c in range(nch):
                    sl = slice(c * 512, (c + 1) * 512)
                    a_ps = pbc.tile([P, 512], F32, tag="bc")
                    c_ps = pbc.tile([P, 512], F32, tag="bc")
                    nc.tensor.matmul(a_ps[:], gb_row[0:1, osl],
                                     rstd_bf[:, sl], start=True, stop=True)
                    nc.tensor.matmul(c_ps[:], gb_row[0:2, osl], mr[:, sl],
                                     start=True, stop=True)
                    t = tmp.tile([P, 512], F32, tag="t512")
                    nc.vector.tensor_tensor(t[:], x_f32[:, o, sl], a_ps[:],
                                            MUL)
                    nc.vector.tensor_tensor(h_out[:, o, sl], t[:], c_ps[:],
                                            ADD)


def build_program():
    nc = bacc.Bacc("TRN2", target_bir_lowering=False, debug=False)

    xT_d = nc.dram_tensor("xT", [P, EO, S], F32, kind="ExternalInput")
    xqT_d = nc.dram_tensor("xqT", [P, EO, TOK], F32, kind="ExternalInput")
    wq_d = nc.dram_tensor("wq", [P, EO, E], BF16, kind="ExternalInput")
    wk_d = nc.dram_tensor("wk", [P, EO, E], BF16, kind="ExternalInput")
    wv_d = nc.dram_tensor("wv", [P, EO, E], BF16, kind="ExternalInput")
    wp_d = nc.dram_tensor("wp", [P, EO, E], BF16, kind="ExternalInput")
    w1_d = nc.dram_tensor("w1", [P, EO, FF], BF16, kind="ExternalInput")
    w2_d = nc.dram_tensor("w2", [P, FO, E], BF16, kind="ExternalInput")
    bias_d = nc.dram_tensor("biases", [P, FO + 2 * EO], F32,
                            kind="ExternalInput")  # bp | b1 | b2
    g1_d = nc.dram_tensor("g1b1", [2, E], BF16, kind="ExternalInput")
    g2_d = nc.dram_tensor("g2b2", [2, E], BF16, kind="ExternalInput")
    mask_d = nc.dram_tensor("mask", [P, 4, GQ], BF16, kind="ExternalInput")
    out_d = nc.dram_tensor("outT", [P, EO, TOK], F32, kind="ExternalOutput")

    with tile.TileContext(nc) as tc:
        const = tc.alloc_tile_pool(name="const", bufs=1)
        tmp = tc.alloc_tile_pool(name="tmp", bufs=3)
        pools = {"tmp": tmp}

        ones_f = const.tile([P, 1], F32)
        nc.vector.memset(ones_f[:], 1.0)
        ones_b = const.tile([P, 64], BF16)   # all-ones; rows usable anywhere
        nc.vector.memset(ones_b[:], 1.0)
        pools["ones_f"] = ones_f
        pools["ones_b"] = ones_b

        gb1 = const.tile([2, E], BF16)
        nc.sync.dma_start(gb1[:], g1_d.ap())
        gb2 = const.tile([2, E], BF16)
        nc.sync.dma_start(gb2[:], g2_d.ap())
        bias_sb = const.tile([P, FO + 2 * EO], F32)
        nc.sync.dma_start(bias_sb[:], bias_d.ap())
        bp_pp = bias_sb[:, 0:EO]
        b1_pp = bias_sb[:, EO:EO + FO]
        b2_pp = bias_sb[:, EO + FO:EO + FO + EO]
        mask_sb = const.tile([P, 4, GQ], BF16)
        nc.sync.dma_start(mask_sb[:], mask_d.ap())

        xq_pool = tc.alloc_tile_pool(name="xq", bufs=1)
        xq_sb = xq_pool.tile([P, EO, TOK], F32)
        nc.sync.dma_start(xq_sb[:], xqT_d.ap())

        # ---------------- LN1 ----------------------------------------
        hkv_pool = tc.alloc_tile_pool(name="hkv", bufs=1, side="right")
        h_kv = hkv_pool.tile([P, EO, S], BF16)
        h_q = hkv_pool.tile([P, EO, TOK], BF16)
        with tc.tile_pool(name="xfull", bufs=1, side="right") as xp:
            x_sb = xp.tile([P, EO, S], F32)
            nc.sync.dma_start(x_sb[:], xT_d.ap())
            _emit_ln(nc, tc, pools, x_sb, gb1, S, h_kv, "l1")
        _emit_ln(nc, tc, pools, xq_sb, gb1, TOK, h_q, "l1q")

        # ---------------- QKV projections -----------------------------
        kvq_pool = tc.alloc_tile_pool(name="kvq", bufs=1)
        k_sb = kvq_pool.tile([P, EO, S], BF16)
        q_sb = kvq_pool.tile([P, EO, TOK], BF16)
        v_sb = kvq_pool.tile([P, KT, H, DK + 1], BF16)
        # ones column at col DK (used by even heads' fused row-sum)
        nc.vector.memset(v_sb[:, :, :, DK], 1.0)

        with tc.tile_pool(name="wkt", bufs=24) as wkp, \
             tc.tile_pool(name="wvt", bufs=1) as wvp, \
             tc.tile_pool(name="qkvps", bufs=4, space="PSUM") as pqkv:
            for kk in range(EO):
                csl = slice(kk * P, (kk + 1) * P)
                wkts = []
                wqts = []
                for o in range(EO):
                    wkt = wkp.tile([P, P], BF16, tag="wt")
                    nc.sync.dma_start(wkt[:], wk_d.ap()[:, o, csl])
                    wkts.append(wkt)
                for o in range(EO):
                    wqt = wkp.tile([P, P], BF16, tag="wt")
                    nc.sync.dma_start(wqt[:], wq_d.ap()[:, o, csl])
                    wqts.append(wqt)
                for c in range(CS):
                    sl = slice(c * 512, (c + 1) * 512)
                    ps = pqkv.tile([P, 512], F32, tag="proj")
                    for o in range(EO):
                        nc.tensor.matmul(ps[:], wkts[o][:], h_kv[:, o, sl],
                                         start=(o == 0), stop=(o == EO - 1))
                    nc.vector.tensor_copy(k_sb[:, kk, sl], ps[:])
                for c in range(CT):
                    sl = slice(c * 512, (c + 1) * 512)
                    ps = pqkv.tile([P, 512], F32, tag="proj")
                    for o in range(EO):
                        nc.tensor.matmul(ps[:], wqts[o][:], h_q[:, o, sl],
                                         start=(o == 0), stop=(o == EO - 1))
                    nc.vector.tensor_copy(q_sb[:, kk, sl], ps[:])
            # V in natural layout: lhsT = activations, rhs = Wv columns
            for dc in range(2):
                sl = slice(dc * 512, (dc + 1) * 512)
                wvt = wvp.tile([P, EO, 512], BF16, tag="wvt")
                nc.sync.dma_start(wvt[:], wv_d.ap()[:, :, sl])
                for kt in range(KT):
                    tsl = slice(kt * P, (kt + 1) * P)
                    ps = pqkv.tile([P, 512], F32, tag="proj")
                    for o in range(EO):
                        nc.tensor.matmul(ps[:], h_kv[:, o, tsl], wvt[:, o, :],
                                         start=(o == 0), stop=(o == EO - 1))
                    nc.vector.tensor_copy(
                        v_sb[:, kt, dc * 8:(dc + 1) * 8, 0:DK],
                        ps.rearrange("p (h d) -> p h d", d=DK))
        hkv_pool.release()

        # ---------------- attention -----------------------------------
        o_pool = tc.alloc_tile_pool(name="oc", bufs=1, side="right")
        o_sb = o_pool.tile([P, EO, TOK], BF16)
        with tc.tile_pool(name="exps", bufs=3, side="right") as exp_pool, \
             tc.tile_pool(name="attsm", bufs=4, side="right") as att_sm, \
             tc.tile_pool(name="attps", bufs=3, space="PSUM") as ps_s, \
             tc.tile_pool(name="attpo", bufs=2, space="PSUM") as ps_o, \
             tc.tile_pool(name="attpc", bufs=2, space="PSUM") as ps_c, \
             tc.tile_pool(name="attpr", bufs=1, space="PSUM") as ps_r:
            for h in range(H):
                kk = h >> 1
                odd = h & 1
                po = odd * DK
                for g in range(NG):
                    nkt = 4 * g + 4
                    qsl = slice(g * GQ, (g + 1) * GQ)
                    es = exp_pool.tile([P, KT, GQ], BF16, tag="exp")
                    for kt in range(nkt):
                        ksl = slice(kt * P, (kt + 1) * P)
                        sc = ps_s.tile([P, GQ], F32, tag="score")
                        nc.tensor.matmul(sc[:], k_sb[po:po + DK, kk, ksl],
                                         q_sb[po:po + DK, kk, qsl],
                                         start=True, stop=True)
                        nc.scalar.activation(es[:, kt, :], sc[:], AF.Exp)
                    nc.vector.tensor_tensor(es[:, 4 * g:nkt, :],
                                            es[:, 4 * g:nkt, :],
                                            mask_sb[:], MUL)
                    osl_o = slice(po, po + DK)
                    oa = ps_o.tile([P, GQ], F32, tag="oacc")
                    if not odd:
                        # fused: rows [0:64]=o, row 64 = exp-score sums
                        for kt in range(nkt):
                            nc.tensor.matmul(oa[0:DK + 1, :],
                                             v_sb[:, kt, h, :], es[:, kt, :],
                                             start=(kt == 0),
                                             stop=(kt == nkt - 1))
                        sums = oa
                        ssl = slice(DK, DK + 1)
                    else:
                        # o at rows [64:128]; separate sums matmul at row 32
                        for kt in range(nkt):
                            nc.tensor.matmul(oa[DK:2 * DK, :],
                                             v_sb[:, kt, h, 0:DK],
                                             es[:, kt, :],
                                             start=(kt == 0),
                                             stop=(kt == nkt - 1))
                        sums = ps_c.tile([P, GQ], F32, tag="sacc")
                        ssl = slice(32, 33)
                        for kt in range(nkt):
                            nc.tensor.matmul(sums[ssl, :], ones_b[:, 0:1],
                                             es[:, kt, :],
                                             start=(kt == 0),
                                             stop=(kt == nkt - 1))
                    rr = att_sm.tile([P, GQ], F32, tag="rr")
                    rrb = att_sm.tile([P, GQ], BF16, tag="rrb")
                    nc.vector.reciprocal(rr[ssl, :], sums[ssl, :])
                    nc.vector.tensor_copy(rrb[ssl, :], rr[ssl, :])
                    rb = ps_r.tile([P, GQ], F32, tag="rb")
                    nc.tensor.matmul(rb[osl_o, :], ones_b[ssl, 0:DK],
                                     rrb[ssl, :], start=True, stop=True)
                    rbs = att_sm.tile([P, GQ], BF16, tag="rbs")
                    nc.scalar.activation(rbs[osl_o, :], rb[osl_o, :], AF.Copy)
                    nc.vector.tensor_tensor(o_sb[osl_o, kk, qsl],
                                            oa[osl_o, :], rbs[osl_o, :], MUL)
        kvq_pool.release()

        # ---------------- proj + residual + LN2 ------------------------
        x2_pool = tc.alloc_tile_pool(name="x2", bufs=1)
        x2_sb = x2_pool.tile([P, EO, TOK], F32)
        h2_sb = x2_pool.tile([P, EO, TOK], BF16)
        with tc.tile_pool(name="wpt", bufs=24) as wpp, \
             tc.tile_pool(name="prps", bufs=4, space="PSUM") as ppr:
            for oo in range(EO):
                osl = slice(oo * P, (oo + 1) * P)
                wpts = []
                for s in range(EO):
                    wpt = wpp.tile([P, P], BF16, tag="wt")
                    nc.sync.dma_start(wpt[:], wp_d.ap()[:, s, osl])
                    wpts.append(wpt)
                for c in range(CT):
                    sl = slice(c * 512, (c + 1) * 512)
                    ps = ppr.tile([P, 512], F32, tag="proj2")
                    for s in range(EO):
                        nc.tensor.matmul(ps[:], wpts[s][:], o_sb[:, s, sl],
                                         start=(s == 0), stop=(s == EO - 1))
                    t = tmp.tile([P, 512], F32, tag="t512")
                    nc.vector.tensor_scalar(t[:], ps[:], bp_pp[:, oo:oo + 1],
                                            None, ADD)
                    nc.vector.tensor_tensor(x2_sb[:, oo, sl], t[:],
                                            xq_sb[:, oo, sl], ADD)
        o_pool.release()
        _emit_ln(nc, tc, pools, x2_sb, gb2, TOK, h2_sb, "l2")

        # ---------------- FFN ------------------------------------------
        with tc.tile_pool(name="relu1", bufs=1) as rp, \
             tc.tile_pool(name="w1s", bufs=3) as w1p, \
             tc.tile_pool(name="ffps", bufs=4, space="PSUM") as pff:
            relu1 = rp.tile([P, FO, TOK], BF16)
            for f in range(FO):
                fsl = slice(f * P, (f + 1) * P)
                w1t = w1p.tile([P, EO, P], BF16, tag="w1t")
                nc.sync.dma_start(w1t[:], w1_d.ap()[:, :, fsl])
                for c in range(CT):
                    sl = slice(c * 512, (c + 1) * 512)
                    ps = pff.tile([P, 512], F32, tag="ff1")
                    for s in range(EO):
                        nc.tensor.matmul(ps[:], w1t[:, s, :], h2_sb[:, s, sl],
                                         start=(s == 0), stop=(s == EO - 1))
                    nc.scalar.activation(relu1[:, f, sl], ps[:], AF.Relu,
                                         bias=b1_pp[:, f:f + 1])
            with tc.tile_pool(name="w2s", bufs=36) as w2p, \
                 tc.tile_pool(name="outs", bufs=4) as outp:
                for oo in range(EO):
                    osl = slice(oo * P, (oo + 1) * P)
                    w2ts = []
                    for s in range(FO):
                        w2t = w2p.tile([P, P], BF16, tag="w2t")
                        nc.sync.dma_start(w2t[:], w2_d.ap()[:, s, osl])
                        w2ts.append(w2t)
                    for c in range(CT):
                        sl = slice(c * 512, (c + 1) * 512)
                        ps = pff.tile([P, 512], F32, tag="ff2")
                        for s in range(FO):
                            nc.tensor.matmul(ps[:], w2ts[s][:],
                                             relu1[:, s, sl],
                                             start=(s == 0),
                                             stop=(s == FO - 1))
                        t = tmp.tile([P, 512], F32, tag="t512")
                        nc.vector.tensor_scalar(t[:], ps[:],
                                                b2_pp[:, oo:oo + 1], None, ADD)
                        ot = outp.tile([P, 512], F32, tag="ot")
                        nc.vector.tensor_tensor(ot[:], t[:],
                                                x2_sb[:, oo, sl], ADD)
                        nc.sync.dma_start(out_d.ap()[:, oo, sl], ot[:])
        x2_pool.release()
        xq_pool.release()
        tmp.release()
        const.release()

    nc.compile()
    return nc


def _feat_tile(w, np_dtype):
    """[E_in, N] row-major -> [128, E_in//128, N] (partition, subtile, col)."""
    ei, n = w.shape
    return np.ascontiguousarray(
        w.reshape(ei // P, P, n).transpose(1, 0, 2)).astype(np_dtype)


def _pp(vec):
    """[N] -> [128, N//128] per-partition layout."""
    n = vec.shape[0]
    return np.ascontiguousarray(vec.reshape(n // P, P).T).astype(np.float32)


def _prepare_in_maps(inputs):
    return _make_in_maps(**{k: np.asarray(v) for k, v in inputs.items()})


def _make_in_maps(x, Wq, Wk, Wv, Wp, bp, W1, b1, W2, b2,
                  ln1_g, ln1_b, ln2_g, ln2_b):
    x = np.asarray(x, np.float32)
    scale = 1.0 / np.sqrt(np.float32(E))
    wq_all = np.asarray(Wq, np.float32).transpose(1, 0, 2).reshape(E, H * DK) * scale
    wk_all = np.asarray(Wk, np.float32).transpose(1, 0, 2).reshape(E, H * DK)
    wv_all = np.asarray(Wv, np.float32).transpose(1, 0, 2).reshape(E, H * DK)

    biases = np.concatenate([
        _pp(np.asarray(bp, np.float32)),
        _pp(np.asarray(b1, np.float32)),
        _pp(np.asarray(b2, np.float32))], axis=1)

    shared = {
        "wq": _feat_tile(wq_all, BF),
        "wk": _feat_tile(wk_all, BF),
        "wv": _feat_tile(wv_all, BF),
        "wp": _feat_tile(np.asarray(Wp, np.float32), BF),
        "w1": _feat_tile(np.asarray(W1, np.float32), BF),
        "w2": _feat_tile(np.asarray(W2, np.float32), BF),
        "biases": biases,
        "g1b1": np.stack([np.asarray(ln1_g), np.asarray(ln1_b)]).astype(BF),
        "g2b2": np.stack([np.asarray(ln2_g), np.asarray(ln2_b)]).astype(BF),
    }

    kappa = np.arange(512)[:, None]
    r = np.arange(GQ)[None, :]
    masks = []
    for j in range(2):
        m = (kappa <= 2 * r + j).astype(np.float32)
        masks.append(np.ascontiguousarray(
            m.reshape(4, P, GQ).transpose(1, 0, 2)).astype(BF))

    in_maps = []
    for c in range(8):
        b, j = c // 2, c % 2
        xbT = np.ascontiguousarray(x[b].T)              # [E, S]
        xqT = np.ascontiguousarray(x[b][j::2].T)        # [E, TOK]
        m = dict(shared)
        m["xT"] = np.ascontiguousarray(
            xbT.reshape(EO, P, S).transpose(1, 0, 2))
        m["xqT"] = np.ascontiguousarray(
            xqT.reshape(EO, P, TOK).transpose(1, 0, 2))
        m["mask"] = masks[j]
        in_maps.append(m)
    return in_maps


def kernel(x, Wq, Wk, Wv, Wp, bp, W1, b1, W2, b2, ln1_g, ln1_b, ln2_g, ln2_b):
    global _PROG
    if _PROG is None:
        _PROG = build_program()
    nc = _PROG

    in_maps = _make_in_maps(x, Wq, Wk, Wv, Wp, bp, W1, b1, W2, b2,
                            ln1_g, ln1_b, ln2_g, ln2_b)
    res = bass_utils.run_bass_kernel_spmd(nc, in_maps, core_ids=list(range(8)))

    out = np.empty((4, S, E), np.float32)
    for c in range(8):
        b, j = c // 2, c % 2
        oT = res.results[c]["outT"]                     # [128, EO, TOK]
        out[b, j::2, :] = oT.transpose(1, 0, 2).reshape(E, TOK).T
    return out
